# revision 13
# baseline (speedup 1.0000x reference)
"""PointPillarScatter Trainium2 kernel (v2).

Strategy: shard by (batch, y-half) -> 8 cores, each producing a
[64, 107136] channel-major slab of the BEV grid, laid out on device as
[128, 53568]: two 256-cell tiles are stacked in the partition dim.

For each "pair" (two adjacent 256-cell tiles A and B, 512 cells), one
fp16 matmul with a block-diagonal lhsT produces all 128 PSUM partitions:

    lhsT [2K, 128]: rows 0:K   = A-pillar features in cols 0:64
                    rows K:2K  = B-pillar features in cols 64:128
    rhs  [2K, 256]: rows 0:K   = onehot(A offsets), rows K:2K = onehot(B)
    psum [128, 256]: rows 0:64 = channels of A cells, 64:128 = B cells

The onehot is built per pair by one DVE tensor_scalar is_equal in fp16
(4x mode: f32 per-partition scalar offsets against a packed fp16 iota
row). Only the nonzero blocks of lhsT are DMA'd from HBM (featA/featB,
2.6 MB per core); the zero blocks are memset once per buffer in the
prologue. PSUM->SBUF copies and HBM stores move [128, n] tiles, so
engine time per cell is half of a [64, n] layout. Host does last-wins
dedup, bucketing, fp16 conversion, and the final de-interleave of the
[128, 53568] slabs.
"""

import numpy as np

B, C, NY, NX = 4, 64, 496, 432
CELLS_B = NY * NX            # 214272 cells per batch
HALF = CELLS_B // 2          # 107136 cells per core slab
N_CORES = 8
TILE = 256                   # cells per onehot block
MAIN_PAIRS = 209             # pairs of full 256-cell blocks (= 107008 cells)
PAIRS = MAIN_PAIRS + 1       # + tail pair: 2 blocks of 64 cells
TAIL_COLS = 64
OUT_COLS = MAIN_PAIRS * TILE + TAIL_COLS  # 53568
K_PAD = 48                   # max pillars per 256-cell block (46 measured)
PAIRS_PER_PSUM = 8           # psum tile [128, 2048]
PAIRS_PER_STAGE = 16         # stage tile [128, 4096]
N_FULL_STAGE = 13            # stages 0..12: 16 pairs each (208 pairs)
# stage 13: pair 208 (256 cols) + tail pair 209 (64 cols) = 320 cols


def _host_prep(pf, vc):
    """Dedup (last-wins), shard, bucket into (core, pair, block) and pad.

    Returns featA, featB [N_CORES, K, PAIRS*64] f16 (the nonzero blocks
            of the block-diagonal lhsT),
            offs  [N_CORES, 2K, PAIRS] f32 (pad = -1),
            iota  [2K, 256] f16,
            K (block K_pad; 2K <= 128 required for the HW path).
    """
    pf = np.asarray(pf, dtype=np.float32)
    vc = np.asarray(vc)
    b = vc[:, 0].astype(np.int64)
    y = vc[:, 2].astype(np.int64)
    x = vc[:, 3].astype(np.int64)
    cell = y * NX + x
    key = b * CELLS_B + cell

    # last occurrence of each key wins (matches reference scatter)
    u, idx_rev = np.unique(key[::-1], return_index=True)
    winners = (len(key) - 1) - idx_rev

    wb = u // CELLS_B
    wc = u % CELLS_B
    h = (wc >= HALF).astype(np.int64)
    core = wb * 2 + h
    cl = wc - h * HALF                      # 0..HALF-1 within slab

    main = cl < MAIN_PAIRS * 2 * TILE
    pair = np.where(main, cl // (2 * TILE), MAIN_PAIRS)
    j = np.where(main, cl % (2 * TILE), cl - MAIN_PAIRS * 2 * TILE)
    blk_sz = np.where(pair < MAIN_PAIRS, TILE, TAIL_COLS)
    blk = (j >= blk_sz).astype(np.int64)    # 0 = A, 1 = B
    off = j - blk * blk_sz                  # offset within block

    gkey = (core * PAIRS + pair) * 2 + blk
    order = np.argsort(gkey, kind="stable")
    gk_s = gkey[order]
    starts = np.r_[0, np.flatnonzero(np.diff(gk_s)) + 1]
    counts = np.diff(np.r_[starts, len(gk_s)])
    K = max(16, int(np.ceil(counts.max() / 16) * 16))

    rank = np.arange(len(gk_s)) - np.repeat(starts, counts)
    w_s = winners[order]
    core_s = core[order]
    pair_s = pair[order]
    blk_s = blk[order]
    off_s = off[order]

    feat = np.zeros((N_CORES, 2, K, PAIRS, 64), np.float16)
    offs = np.full((N_CORES, 2 * K, PAIRS), -1.0, np.float32)
    feat[core_s, blk_s, rank, pair_s, :] = pf[w_s].astype(np.float16)
    offs[core_s, blk_s * K + rank, pair_s] = off_s

    iota = np.broadcast_to(
        np.arange(TILE, dtype=np.float16)[None, :], (2 * K, TILE)
    ).copy()
    featA = feat[:, 0].reshape(N_CORES, K, PAIRS * 64)
    featB = feat[:, 1].reshape(N_CORES, K, PAIRS * 64)
    return featA, featB, offs, iota, K


def _unshuffle(out_dev):
    """[N_CORES, 128, OUT_COLS] -> [B, C, NY, NX]."""
    full = np.empty((B, C, CELLS_B), np.float32)
    for core in range(N_CORES):
        bb, hh = core // 2, core % 2
        od = out_dev[core]
        slab = np.empty((C, HALF), np.float32)
        m = MAIN_PAIRS * TILE
        s = slab[:, : 2 * m].reshape(C, MAIN_PAIRS, 2 * TILE)
        s[:, :, :TILE] = od[:64, :m].reshape(C, MAIN_PAIRS, TILE)
        s[:, :, TILE:] = od[64:, :m].reshape(C, MAIN_PAIRS, TILE)
        slab[:, 2 * m : 2 * m + TAIL_COLS] = od[:64, m:]
        slab[:, 2 * m + TAIL_COLS :] = od[64:, m:]
        full[bb, :, hh * HALF : (hh + 1) * HALF] = slab
    return full.reshape(B, C, NY, NX)


def _sim_core(featA_c, featB_c, offs_c, K):
    """Numpy simulation of one core's device program (for validation)."""
    out = np.zeros((128, OUT_COLS), np.float32)
    fA = featA_c.reshape(K, PAIRS, 64).astype(np.float32)
    fB = featB_c.reshape(K, PAIRS, 64).astype(np.float32)
    for t in range(PAIRS):
        n = TILE if t < MAIN_PAIRS else TAIL_COLS
        oh = (offs_c[:, t : t + 1] == np.arange(n)[None, :]).astype(np.float32)
        lhsT = np.zeros((2 * K, 128), np.float32)
        lhsT[:K, :64] = fA[:, t, :]
        lhsT[K:, 64:] = fB[:, t, :]
        lo = t * TILE
        out[:, lo : lo + n] = lhsT.T @ oh
    return out


def _build_bass(K, dynamic=False, load_ring="scalar", split_feat=True, onehot="ts"):
    import concourse.bacc as bacc
    import concourse.tile as tile
    from concourse import mybir
    from contextlib import ExitStack

    f32 = mybir.dt.float32
    f16 = mybir.dt.float16
    i32 = mybir.dt.int32
    K2 = 2 * K
    nc = bacc.Bacc("TRN2", target_bir_lowering=False, debug=False)

    if split_feat:
        featA = nc.dram_tensor("featA", [K, PAIRS * 64], f16, kind="ExternalInput")
        featB = nc.dram_tensor("featB", [K, PAIRS * 64], f16, kind="ExternalInput")
    else:
        featT = nc.dram_tensor(
            "featT", [K2, PAIRS * 128], f16, kind="ExternalInput"
        )
    offs = nc.dram_tensor(
        "offs", [K2, PAIRS], f32 if onehot == "ts" else f16, kind="ExternalInput"
    )
    iota = nc.dram_tensor("iota", [K2, TILE], f16, kind="ExternalInput")
    if dynamic:
        reps = nc.dram_tensor("reps", [1, 1], i32, kind="ExternalInput")
    out = nc.dram_tensor("out", [128, OUT_COLS], f32, kind="ExternalOutput")

    N_FEAT_BUFS = 3

    with tile.TileContext(nc) as tc, ExitStack() as ctx:
        const_p = ctx.enter_context(tc.tile_pool(name="const", bufs=1))
        feat_p = ctx.enter_context(tc.tile_pool(name="feat", bufs=N_FEAT_BUFS))
        oh_p = ctx.enter_context(tc.tile_pool(name="oh", bufs=16))
        ps_p = ctx.enter_context(tc.tile_pool(name="ps", bufs=2, space="PSUM"))
        st_p = ctx.enter_context(tc.tile_pool(name="st", bufs=3))

        iota_t = const_p.tile([K2, TILE], f16)
        nc.sync.dma_start(out=iota_t[:], in_=iota[:, :])
        off_t = const_p.tile([K2, PAIRS], f32 if onehot == "ts" else f16)
        nc.sync.dma_start(out=off_t[:], in_=offs[:, :])
        if dynamic:
            rt = const_p.tile([1, 1], i32)
            nc.sync.dma_start(out=rt[:], in_=reps[:, :])
            r_val = nc.values_load(
                rt[:], min_val=1, max_val=1 << 20, skip_runtime_bounds_check=True
            )

        # prologue: zero the anti-diagonal blocks of every feat buffer once;
        # the per-chunk DMAs below only ever write the diagonal blocks, so
        # the zeros persist across all loop iterations / buffer reuse.
        if split_feat:
            for _ in range(N_FEAT_BUFS):
                zt = feat_p.tile([K2, PAIRS_PER_STAGE * 128], f16, tag="feat")
                nc.vector.memset(zt[:], 0)

        load_eng = {"scalar": nc.scalar, "sync": nc.sync, "gpsimd": nc.gpsimd}[
            load_ring
        ]

        def body():
            for g in range(N_FULL_STAGE + 1):
                p0 = g * PAIRS_PER_STAGE
                n_pairs = PAIRS_PER_STAGE if g < N_FULL_STAGE else PAIRS - p0
                g_cols = (
                    PAIRS_PER_STAGE * TILE
                    if g < N_FULL_STAGE
                    else (n_pairs - 1) * TILE + TAIL_COLS
                )
                c0 = p0 * TILE

                # this stage's lhsT chunk: load only the nonzero blocks
                # (strided into the block-diagonal layout)
                feat_chunk = feat_p.tile(
                    [K2, PAIRS_PER_STAGE * 128], f16, tag="feat"
                )
                fc = feat_chunk[:, : n_pairs * 128]
                if split_feat:
                    load_eng.dma_start(
                        out=fc[:K].rearrange("k (p c) -> k p c", p=n_pairs)[
                            :, :, :64
                        ],
                        in_=featA[:, p0 * 64 : (p0 + n_pairs) * 64],
                    )
                    load_eng.dma_start(
                        out=fc[K:].rearrange("k (p c) -> k p c", p=n_pairs)[
                            :, :, 64:
                        ],
                        in_=featB[:, p0 * 64 : (p0 + n_pairs) * 64],
                    )
                else:
                    load_eng.dma_start(
                        out=fc[:],
                        in_=featT[:, p0 * 128 : (p0 + n_pairs) * 128],
                    )

                stage = st_p.tile([128, g_cols], f32, tag="st")
                n_ps = (n_pairs + PAIRS_PER_PSUM - 1) // PAIRS_PER_PSUM
                for q in range(n_ps):
                    qp0 = q * PAIRS_PER_PSUM
                    q_pairs = min(PAIRS_PER_PSUM, n_pairs - qp0)
                    q_cols = sum(
                        TILE if p0 + qp0 + i < MAIN_PAIRS else TAIL_COLS
                        for i in range(q_pairs)
                    )
                    psum = ps_p.tile([128, q_cols], f32, tag="ps")
                    col = 0
                    for i in range(q_pairs):
                        t = p0 + qp0 + i
                        n = TILE if t < MAIN_PAIRS else TAIL_COLS
                        oh = oh_p.tile([K2, n], f16, tag="oh")
                        if onehot == "ts":
                            nc.vector.tensor_scalar(
                                out=oh[:],
                                in0=iota_t[:, :n],
                                scalar1=off_t[:, t : t + 1],
                                scalar2=None,
                                op0=mybir.AluOpType.is_equal,
                            )
                        else:
                            nc.vector.tensor_tensor(
                                out=oh[:],
                                in0=off_t[:, t : t + 1].to_broadcast([K2, n]),
                                in1=iota_t[:, :n],
                                op=mybir.AluOpType.is_equal,
                            )
                        j = qp0 + i
                        nc.tensor.matmul(
                            out=psum[:, col : col + n],
                            lhsT=feat_chunk[:, j * 128 : j * 128 + 128],
                            rhs=oh[:],
                            is_transpose=False,
                            start=True,
                            stop=True,
                        )
                        col += n
                    nc.scalar.copy(
                        out=stage[:, qp0 * TILE : qp0 * TILE + q_cols], in_=psum[:]
                    )
                nc.sync.dma_start(out=out[:, c0 : c0 + g_cols], in_=stage[:])

        if dynamic:
            with tc.For_i(0, r_val, 1):
                body()
        else:
            body()

    nc.compile()
    return nc


def _dense_featT(featA, featB, K):
    """[N_CORES, 2K, PAIRS*128] block-diagonal fp16 (for split_feat=False)."""
    featT = np.zeros((N_CORES, 2 * K, PAIRS, 128), np.float16)
    featT[:, :K, :, :64] = featA.reshape(N_CORES, K, PAIRS, 64)
    featT[:, K:, :, 64:] = featB.reshape(N_CORES, K, PAIRS, 64)
    return featT.reshape(N_CORES, 2 * K, PAIRS * 128)


def _make_in_maps(
    featA, featB, offs, iota, dynamic=False, split_feat=True, K=None, onehot="ts"
):
    dense = None if split_feat else _dense_featT(featA, featB, K)
    offs_in = offs if onehot == "ts" else offs.astype(np.float16)
    maps = []
    for c in range(N_CORES):
        if split_feat:
            m = {"featA": featA[c], "featB": featB[c]}
        else:
            m = {"featT": dense[c]}
        m.update({"offs": offs_in[c], "iota": iota})
        if dynamic:
            m["reps"] = np.ones((1, 1), np.int32)
        maps.append(m)
    return maps


def _run(featA, featB, offs, iota, K):
    from concourse.bass_utils import run_bass_kernel_spmd

    nc = _build_bass(K)
    in_maps = _make_in_maps(featA, featB, offs, iota)
    res = run_bass_kernel_spmd(nc, in_maps, core_ids=list(range(N_CORES)))
    out_dev = np.stack([res.results[c]["out"] for c in range(N_CORES)])
    return _unshuffle(out_dev)


def kernel(pillar_features, voxel_coords):
    featA, featB, offs, iota, K = _host_prep(pillar_features, voxel_coords)
    if 2 * K > 128:
        # PE contraction is capped at 128 partitions; with the given input
        # distribution K is 48, so this path is never taken. Correctness
        # safety net only.
        out_dev = np.stack(
            [_sim_core(featA[c], featB[c], offs[c], K) for c in range(N_CORES)]
        )
        return _unshuffle(out_dev)
    return _run(featA, featB, offs, iota, K)


if __name__ == "__main__":
    # numpy-sim self check against last-wins reference
    rng = np.random.default_rng(0)
    n = 100000
    pf = rng.standard_normal((n, 64)).astype(np.float32)
    vc = np.stack(
        [
            rng.integers(0, B, n),
            np.zeros(n, np.int64),
            rng.integers(0, NY, n),
            rng.integers(0, NX, n),
        ],
        axis=1,
    ).astype(np.int64)
    featA, featB, offs, iota, K = _host_prep(pf, vc)
    print("K =", K)
    out_dev = np.stack(
        [_sim_core(featA[c], featB[c], offs[c], K) for c in range(N_CORES)]
    )
    got = _unshuffle(out_dev)
    grid = np.zeros((B * CELLS_B, 64), np.float32)
    flat = vc[:, 0] * CELLS_B + vc[:, 2] * NX + vc[:, 3]
    grid[flat] = pf
    ref = grid.reshape(B, CELLS_B, 64).transpose(0, 2, 1).reshape(B, C, NY, NX)
    err = np.abs(got - ref).max() / np.abs(ref).max()
    print("max rel diff vs f32 last-wins reference:", err)
    assert err < 1e-3, err
    print("numpy sim matches (up to fp16 rounding)")


# revision 24
# speedup vs baseline: 1.1043x; 1.1043x over previous
"""PointPillarScatter Trainium2 kernel (v2).

Strategy: shard by (batch, y-half) -> 8 cores, each producing a
[64, 107136] channel-major slab of the BEV grid, laid out on device as
[128, 53568]: two 256-cell tiles are stacked in the partition dim.

For each "pair" (two adjacent 256-cell tiles A and B, 512 cells), one
fp16 matmul with a block-diagonal lhsT produces all 128 PSUM partitions:

    lhsT [2K, 128]: rows 0:K   = A-pillar features in cols 0:64
                    rows K:2K  = B-pillar features in cols 64:128
    rhs  [2K, 256]: rows 0:K   = onehot(A offsets), rows K:2K = onehot(B)
    psum [128, 256]: rows 0:64 = channels of A cells, 64:128 = B cells

The onehot is built per pair by one DVE tensor_scalar is_equal in fp16
(4x mode: f32 per-partition scalar offsets against a packed fp16 iota
row). Only the nonzero blocks of lhsT are DMA'd from HBM (featA/featB,
2.6 MB per core); the zero blocks are memset once per buffer in the
prologue. PSUM->SBUF copies and HBM stores move [128, n] tiles, so
engine time per cell is half of a [64, n] layout. Host does last-wins
dedup, bucketing, fp16 conversion, and the final de-interleave of the
[128, 53568] slabs.
"""

import numpy as np

B, C, NY, NX = 4, 64, 496, 432
CELLS_B = NY * NX            # 214272 cells per batch
HALF = CELLS_B // 2          # 107136 cells per core slab
N_CORES = 8
TILE = 256                   # cells per onehot block
MAIN_PAIRS = 209             # pairs of full 256-cell blocks (= 107008 cells)
PAIRS = MAIN_PAIRS + 1       # + tail pair: 2 blocks of 64 cells
TAIL_COLS = 64
OUT_COLS = MAIN_PAIRS * TILE + TAIL_COLS  # 53568
K_PAD = 48                   # max pillars per 256-cell block (46 measured)
PAIRS_PER_PSUM = 8           # psum tile [128, 2048]
PAIRS_PER_STAGE = 16         # stage tile [128, 4096]
N_FULL_STAGE = 13            # stages 0..12: 16 pairs each (208 pairs)
# stage 13: pair 208 (256 cols) + tail pair 209 (64 cols) = 320 cols


def _stage_plan(ramp=True):
    """List of (p0, n_pairs) stages covering all PAIRS. Small stages first
    (including the ragged tail stage) shorten the pipeline fill before the
    first HBM store and let the body end on a full-size store."""
    if ramp:
        plan = [(208, 2), (0, 2), (2, 2), (4, 4), (8, 8)] + [
            (16 + 16 * i, 16) for i in range(12)
        ]
    else:
        plan = [(16 * i, 16) for i in range(13)] + [(208, 2)]
    assert sorted(p0 for p0, _ in plan)
    assert sum(n for _, n in plan) == PAIRS
    return plan


def _stage_cols(p0, n_pairs):
    return sum(TILE if p0 + i < MAIN_PAIRS else TAIL_COLS for i in range(n_pairs))


def _featAB(featA, featB, K, plan):
    """Per-stage interleaved [N_CORES, K, 2*PAIRS*64] fp16: for each stage,
    its A block then its B block, contiguously."""
    fA = featA.reshape(N_CORES, K, PAIRS, 64)
    fB = featB.reshape(N_CORES, K, PAIRS, 64)
    out = np.empty((N_CORES, K, 2 * PAIRS * 64), np.float16)
    col = 0
    for p0, n in plan:
        w = n * 64
        out[:, :, col : col + w] = fA[:, :, p0 : p0 + n].reshape(N_CORES, K, w)
        out[:, :, col + w : col + 2 * w] = fB[:, :, p0 : p0 + n].reshape(
            N_CORES, K, w
        )
        col += 2 * w
    return out


def _host_prep(pf, vc):
    """Dedup (last-wins), shard, bucket into (core, pair, block) and pad.

    Returns featA, featB [N_CORES, K, PAIRS*64] f16 (the nonzero blocks
            of the block-diagonal lhsT),
            offs  [N_CORES, 2K, PAIRS] f32 (pad = -1),
            iota  [2K, 256] f16,
            K (block K_pad; 2K <= 128 required for the HW path).
    """
    pf = np.asarray(pf, dtype=np.float32)
    vc = np.asarray(vc)
    b = vc[:, 0].astype(np.int64)
    y = vc[:, 2].astype(np.int64)
    x = vc[:, 3].astype(np.int64)
    cell = y * NX + x
    key = b * CELLS_B + cell

    # last occurrence of each key wins (matches reference scatter)
    u, idx_rev = np.unique(key[::-1], return_index=True)
    winners = (len(key) - 1) - idx_rev

    wb = u // CELLS_B
    wc = u % CELLS_B
    h = (wc >= HALF).astype(np.int64)
    core = wb * 2 + h
    cl = wc - h * HALF                      # 0..HALF-1 within slab

    main = cl < MAIN_PAIRS * 2 * TILE
    pair = np.where(main, cl // (2 * TILE), MAIN_PAIRS)
    j = np.where(main, cl % (2 * TILE), cl - MAIN_PAIRS * 2 * TILE)
    blk_sz = np.where(pair < MAIN_PAIRS, TILE, TAIL_COLS)
    blk = (j >= blk_sz).astype(np.int64)    # 0 = A, 1 = B
    off = j - blk * blk_sz                  # offset within block

    gkey = (core * PAIRS + pair) * 2 + blk
    order = np.argsort(gkey, kind="stable")
    gk_s = gkey[order]
    starts = np.r_[0, np.flatnonzero(np.diff(gk_s)) + 1]
    counts = np.diff(np.r_[starts, len(gk_s)])
    K = max(16, int(np.ceil(counts.max() / 16) * 16))

    rank = np.arange(len(gk_s)) - np.repeat(starts, counts)
    w_s = winners[order]
    core_s = core[order]
    pair_s = pair[order]
    blk_s = blk[order]
    off_s = off[order]

    KB = -(-K // 32) * 32            # B-block partition offset (32-aligned)
    feat = np.zeros((N_CORES, 2, K, PAIRS, 64), np.float16)
    offs = np.full((N_CORES, KB + K, PAIRS), -1.0, np.float32)
    feat[core_s, blk_s, rank, pair_s, :] = pf[w_s].astype(np.float16)
    offs[core_s, blk_s * KB + rank, pair_s] = off_s

    iota = np.broadcast_to(
        np.arange(TILE, dtype=np.float16)[None, :], (KB + K, TILE)
    ).copy()
    featA = feat[:, 0].reshape(N_CORES, K, PAIRS * 64)
    featB = feat[:, 1].reshape(N_CORES, K, PAIRS * 64)
    return featA, featB, offs, iota, K


def _unshuffle(out_dev):
    """[N_CORES, 128, OUT_COLS] -> [B, C, NY, NX]."""
    full = np.empty((B, C, CELLS_B), np.float32)
    for core in range(N_CORES):
        bb, hh = core // 2, core % 2
        od = out_dev[core]
        slab = np.empty((C, HALF), np.float32)
        m = MAIN_PAIRS * TILE
        s = slab[:, : 2 * m].reshape(C, MAIN_PAIRS, 2 * TILE)
        s[:, :, :TILE] = od[:64, :m].reshape(C, MAIN_PAIRS, TILE)
        s[:, :, TILE:] = od[64:, :m].reshape(C, MAIN_PAIRS, TILE)
        slab[:, 2 * m : 2 * m + TAIL_COLS] = od[:64, m:]
        slab[:, 2 * m + TAIL_COLS :] = od[64:, m:]
        full[bb, :, hh * HALF : (hh + 1) * HALF] = slab
    return full.reshape(B, C, NY, NX)


def _sim_core(featA_c, featB_c, offs_c, K):
    """Numpy simulation of one core's device program (for validation)."""
    KB = -(-K // 32) * 32
    out = np.zeros((128, OUT_COLS), np.float32)
    fA = featA_c.reshape(K, PAIRS, 64).astype(np.float32)
    fB = featB_c.reshape(K, PAIRS, 64).astype(np.float32)
    for t in range(PAIRS):
        n = TILE if t < MAIN_PAIRS else TAIL_COLS
        oh = (offs_c[:, t : t + 1] == np.arange(n)[None, :]).astype(np.float32)
        lhsT = np.zeros((KB + K, 128), np.float32)
        lhsT[:K, :64] = fA[:, t, :]
        lhsT[KB:, 64:] = fB[:, t, :]
        lo = t * TILE
        out[:, lo : lo + n] = lhsT.T @ oh
    return out


def _build_bass(K, dynamic=False, load_ring="scalar", split_feat="assemble", onehot="ts"):
    import concourse.bacc as bacc
    import concourse.tile as tile
    from concourse import mybir
    from contextlib import ExitStack

    f32 = mybir.dt.float32
    f16 = mybir.dt.float16
    i32 = mybir.dt.int32
    KB = -(-K // 32) * 32
    K2 = KB + K
    nc = bacc.Bacc("TRN2", target_bir_lowering=False, debug=False)

    # split_feat: "dense"    - host-built block-diag featT (5.2 MB loads)
    #             "dma"      - featA/featB strided-DMA'd into the diag blocks
    #             "assemble" - featA/featB contiguous loads + DVE copies
    if split_feat is True:
        split_feat = "dma"
    elif split_feat is False:
        split_feat = "dense"
    if split_feat == "assemble":
        featAB = nc.dram_tensor(
            "featAB", [K, 2 * PAIRS * 64], f16, kind="ExternalInput"
        )
    elif split_feat == "dma":
        featA = nc.dram_tensor("featA", [K, PAIRS * 64], f16, kind="ExternalInput")
        featB = nc.dram_tensor("featB", [K, PAIRS * 64], f16, kind="ExternalInput")
    else:
        featT = nc.dram_tensor(
            "featT", [K2, PAIRS * 128], f16, kind="ExternalInput"
        )
    offs = nc.dram_tensor(
        "offs", [K2, PAIRS], f32 if onehot == "ts" else f16, kind="ExternalInput"
    )
    iota = nc.dram_tensor("iota", [K2, TILE], f16, kind="ExternalInput")
    if dynamic:
        reps = nc.dram_tensor("reps", [1, 1], i32, kind="ExternalInput")
    out = nc.dram_tensor("out", [128, OUT_COLS], f32, kind="ExternalOutput")

    N_FEAT_BUFS = 3

    with tile.TileContext(nc) as tc, ExitStack() as ctx:
        const_p = ctx.enter_context(tc.tile_pool(name="const", bufs=1))
        feat_p = ctx.enter_context(tc.tile_pool(name="feat", bufs=N_FEAT_BUFS))
        oh_p = ctx.enter_context(tc.tile_pool(name="oh", bufs=16))
        ps_p = ctx.enter_context(tc.tile_pool(name="ps", bufs=2, space="PSUM"))
        st_p = ctx.enter_context(tc.tile_pool(name="st", bufs=4))
        ld_p = ctx.enter_context(tc.tile_pool(name="ld", bufs=N_FEAT_BUFS))

        iota_t = const_p.tile([K2, TILE], f16)
        nc.sync.dma_start(out=iota_t[:], in_=iota[:, :])
        off_t = const_p.tile([K2, PAIRS], f32 if onehot == "ts" else f16)
        nc.sync.dma_start(out=off_t[:], in_=offs[:, :])
        if dynamic:
            rt = const_p.tile([1, 1], i32)
            nc.sync.dma_start(out=rt[:], in_=reps[:, :])
            r_val = nc.values_load(
                rt[:], min_val=1, max_val=1 << 20, skip_runtime_bounds_check=True
            )

        # prologue: zero the anti-diagonal blocks of every feat buffer once;
        # the per-chunk DMAs below only ever write the diagonal blocks, so
        # the zeros persist across all loop iterations / buffer reuse.
        plan = _stage_plan(ramp=True)
        max_pairs = max(n for _, n in plan)
        if split_feat != "dense":
            for _ in range(N_FEAT_BUFS):
                zt = feat_p.tile([K2, max_pairs * 128], f16, tag="feat")
                nc.vector.memset(zt[:], 0)

        load_eng = {"scalar": nc.scalar, "sync": nc.sync, "gpsimd": nc.gpsimd}[
            load_ring
        ]

        def body():
            ab_col = 0
            for p0, n_pairs in plan:
                g_cols = _stage_cols(p0, n_pairs)
                c0 = p0 * TILE

                feat_chunk = feat_p.tile([K2, max_pairs * 128], f16, tag="feat")
                fc = feat_chunk[:, : n_pairs * 128]
                nA = n_pairs * 64
                if split_feat == "dma":
                    load_eng.dma_start(
                        out=fc[:K].rearrange("k (p c) -> k p c", p=n_pairs)[
                            :, :, :64
                        ],
                        in_=featA[:, p0 * 64 : (p0 + n_pairs) * 64],
                    )
                    load_eng.dma_start(
                        out=fc[KB:].rearrange("k (p c) -> k p c", p=n_pairs)[
                            :, :, 64:
                        ],
                        in_=featB[:, p0 * 64 : (p0 + n_pairs) * 64],
                    )
                elif split_feat == "assemble":
                    ld = ld_p.tile([K, 2 * max_pairs * 64], f16, tag="ld")
                    load_eng.dma_start(
                        out=ld[:, : 2 * nA], in_=featAB[:, ab_col : ab_col + 2 * nA]
                    )
                    nc.vector.tensor_scalar(
                        out=fc[:K].rearrange("k (p c) -> k p c", p=n_pairs)[
                            :, :, :64
                        ],
                        in0=ld[:, :nA].rearrange("k (p c) -> k p c", p=n_pairs),
                        scalar1=1.0,
                        scalar2=None,
                        op0=mybir.AluOpType.mult,
                    )
                    nc.vector.tensor_scalar(
                        out=fc[KB:].rearrange("k (p c) -> k p c", p=n_pairs)[
                            :, :, 64:
                        ],
                        in0=ld[:, nA : 2 * nA].rearrange(
                            "k (p c) -> k p c", p=n_pairs
                        ),
                        scalar1=1.0,
                        scalar2=None,
                        op0=mybir.AluOpType.mult,
                    )
                else:
                    load_eng.dma_start(
                        out=fc[:],
                        in_=featT[:, p0 * 128 : (p0 + n_pairs) * 128],
                    )

                stage = st_p.tile([128, g_cols], f32, tag="st")
                n_ps = (n_pairs + PAIRS_PER_PSUM - 1) // PAIRS_PER_PSUM
                for q in range(n_ps):
                    qp0 = q * PAIRS_PER_PSUM
                    q_pairs = min(PAIRS_PER_PSUM, n_pairs - qp0)
                    q_cols = _stage_cols(p0 + qp0, q_pairs)
                    psum = ps_p.tile([128, q_cols], f32, tag="ps")
                    col = 0
                    for i in range(q_pairs):
                        t = p0 + qp0 + i
                        n = TILE if t < MAIN_PAIRS else TAIL_COLS
                        oh = oh_p.tile([K2, n], f16, tag="oh")
                        if onehot == "ts":
                            nc.vector.tensor_scalar(
                                out=oh[:],
                                in0=iota_t[:, :n],
                                scalar1=off_t[:, t : t + 1],
                                scalar2=None,
                                op0=mybir.AluOpType.is_equal,
                            )
                        else:
                            nc.vector.tensor_tensor(
                                out=oh[:],
                                in0=off_t[:, t : t + 1].to_broadcast([K2, n]),
                                in1=iota_t[:, :n],
                                op=mybir.AluOpType.is_equal,
                            )
                        j = qp0 + i
                        nc.tensor.matmul(
                            out=psum[:, col : col + n],
                            lhsT=feat_chunk[:, j * 128 : j * 128 + 128],
                            rhs=oh[:],
                            is_transpose=False,
                            start=True,
                            stop=True,
                        )
                        col += n
                    nc.scalar.copy(
                        out=stage[:, qp0 * TILE : qp0 * TILE + q_cols], in_=psum[:]
                    )
                nc.sync.dma_start(out=out[:, c0 : c0 + g_cols], in_=stage[:])
                ab_col += 2 * nA

        if dynamic:
            with tc.For_i(0, r_val, 1):
                body()
        else:
            body()

    nc.compile()
    return nc


def _dense_featT(featA, featB, K):
    """[N_CORES, KB+K, PAIRS*128] block-diagonal fp16 (for dense loads)."""
    KB = -(-K // 32) * 32
    featT = np.zeros((N_CORES, KB + K, PAIRS, 128), np.float16)
    featT[:, :K, :, :64] = featA.reshape(N_CORES, K, PAIRS, 64)
    featT[:, KB:, :, 64:] = featB.reshape(N_CORES, K, PAIRS, 64)
    return featT.reshape(N_CORES, KB + K, PAIRS * 128)


def _make_in_maps(
    featA, featB, offs, iota, dynamic=False, split_feat="assemble", K=None, onehot="ts"
):
    if split_feat is True:
        split_feat = "dma"
    elif split_feat is False:
        split_feat = "dense"
    dense = _dense_featT(featA, featB, K) if split_feat == "dense" else None
    fab = (
        _featAB(featA, featB, K, _stage_plan(ramp=True))
        if split_feat == "assemble"
        else None
    )
    offs_in = offs if onehot == "ts" else offs.astype(np.float16)
    maps = []
    for c in range(N_CORES):
        if split_feat == "assemble":
            m = {"featAB": fab[c]}
        elif split_feat == "dma":
            m = {"featA": featA[c], "featB": featB[c]}
        else:
            m = {"featT": dense[c]}
        m.update({"offs": offs_in[c], "iota": iota})
        if dynamic:
            m["reps"] = np.ones((1, 1), np.int32)
        maps.append(m)
    return maps


def _run(featA, featB, offs, iota, K):
    from concourse.bass_utils import run_bass_kernel_spmd

    nc = _build_bass(K)
    in_maps = _make_in_maps(featA, featB, offs, iota, K=K)
    res = run_bass_kernel_spmd(nc, in_maps, core_ids=list(range(N_CORES)))
    out_dev = np.stack([res.results[c]["out"] for c in range(N_CORES)])
    return _unshuffle(out_dev)


def kernel(pillar_features, voxel_coords):
    featA, featB, offs, iota, K = _host_prep(pillar_features, voxel_coords)
    if K > 64:
        # PE contraction is capped at 128 partitions; with the given input
        # distribution K is 48, so this path is never taken. Correctness
        # safety net only.
        out_dev = np.stack(
            [_sim_core(featA[c], featB[c], offs[c], K) for c in range(N_CORES)]
        )
        return _unshuffle(out_dev)
    return _run(featA, featB, offs, iota, K)


if __name__ == "__main__":
    # numpy-sim self check against last-wins reference
    rng = np.random.default_rng(0)
    n = 100000
    pf = rng.standard_normal((n, 64)).astype(np.float32)
    vc = np.stack(
        [
            rng.integers(0, B, n),
            np.zeros(n, np.int64),
            rng.integers(0, NY, n),
            rng.integers(0, NX, n),
        ],
        axis=1,
    ).astype(np.int64)
    featA, featB, offs, iota, K = _host_prep(pf, vc)
    print("K =", K)
    out_dev = np.stack(
        [_sim_core(featA[c], featB[c], offs[c], K) for c in range(N_CORES)]
    )
    got = _unshuffle(out_dev)
    grid = np.zeros((B * CELLS_B, 64), np.float32)
    flat = vc[:, 0] * CELLS_B + vc[:, 2] * NX + vc[:, 3]
    grid[flat] = pf
    ref = grid.reshape(B, CELLS_B, 64).transpose(0, 2, 1).reshape(B, C, NY, NX)
    err = np.abs(got - ref).max() / np.abs(ref).max()
    print("max rel diff vs f32 last-wins reference:", err)
    assert err < 1e-3, err
    print("numpy sim matches (up to fp16 rounding)")


# revision 26
# speedup vs baseline: 1.3401x; 1.2136x over previous
"""PointPillarScatter Trainium2 kernel (v2).

Strategy: shard by (batch, y-half) -> 8 cores, each producing a
[64, 107136] channel-major slab of the BEV grid, laid out on device as
[128, 53568]: two 256-cell tiles are stacked in the partition dim.

For each "pair" (two adjacent 256-cell tiles A and B, 512 cells), one
fp16 matmul with a block-diagonal lhsT produces all 128 PSUM partitions:

    lhsT [2K, 128]: rows 0:K   = A-pillar features in cols 0:64
                    rows K:2K  = B-pillar features in cols 64:128
    rhs  [2K, 256]: rows 0:K   = onehot(A offsets), rows K:2K = onehot(B)
    psum [128, 256]: rows 0:64 = channels of A cells, 64:128 = B cells

The onehot is built per pair by one DVE tensor_scalar is_equal in fp16
(4x mode: f32 per-partition scalar offsets against a packed fp16 iota
row). Only the nonzero blocks of lhsT are DMA'd from HBM (featA/featB,
2.6 MB per core); the zero blocks are memset once per buffer in the
prologue. PSUM->SBUF copies and HBM stores move [128, n] tiles, so
engine time per cell is half of a [64, n] layout. Host does last-wins
dedup, bucketing, fp16 conversion, and the final de-interleave of the
[128, 53568] slabs.
"""

import numpy as np

B, C, NY, NX = 4, 64, 496, 432
CELLS_B = NY * NX            # 214272 cells per batch
HALF = CELLS_B // 2          # 107136 cells per core slab
N_CORES = 8
TILE = 256                   # cells per onehot block
MAIN_PAIRS = 209             # pairs of full 256-cell blocks (= 107008 cells)
PAIRS = MAIN_PAIRS + 1       # + tail pair: 2 blocks of 64 cells
TAIL_COLS = 64
OUT_COLS = MAIN_PAIRS * TILE + TAIL_COLS  # 53568
K_PAD = 48                   # max pillars per 256-cell block (46 measured)
PAIRS_PER_PSUM = 8           # psum tile [128, 2048]
PAIRS_PER_STAGE = 16         # stage tile [128, 4096]
N_FULL_STAGE = 13            # stages 0..12: 16 pairs each (208 pairs)
# stage 13: pair 208 (256 cols) + tail pair 209 (64 cols) = 320 cols


def _stage_plan(ramp=True):
    """List of (p0, n_pairs) stages covering all PAIRS. Small stages first
    (including the ragged tail stage) shorten the pipeline fill before the
    first HBM store and let the body end on a full-size store."""
    if ramp:
        plan = [(208, 2), (0, 2), (2, 2), (4, 4), (8, 8)] + [
            (16 + 16 * i, 16) for i in range(12)
        ]
    else:
        plan = [(16 * i, 16) for i in range(13)] + [(208, 2)]
    assert sorted(p0 for p0, _ in plan)
    assert sum(n for _, n in plan) == PAIRS
    return plan


def _stage_cols(p0, n_pairs):
    return sum(TILE if p0 + i < MAIN_PAIRS else TAIL_COLS for i in range(n_pairs))


def _featAB(featA, featB, K, plan):
    """Per-stage interleaved [N_CORES, K, 2*PAIRS*64] fp16: for each stage,
    its A block then its B block, contiguously."""
    fA = featA.reshape(N_CORES, K, PAIRS, 64)
    fB = featB.reshape(N_CORES, K, PAIRS, 64)
    out = np.empty((N_CORES, K, 2 * PAIRS * 64), np.float16)
    col = 0
    for p0, n in plan:
        w = n * 64
        out[:, :, col : col + w] = fA[:, :, p0 : p0 + n].reshape(N_CORES, K, w)
        out[:, :, col + w : col + 2 * w] = fB[:, :, p0 : p0 + n].reshape(
            N_CORES, K, w
        )
        col += 2 * w
    return out


def _host_prep(pf, vc):
    """Dedup (last-wins), shard, bucket into (core, pair, block) and pad.

    Returns featA, featB [N_CORES, K, PAIRS*64] f16 (the nonzero blocks
            of the block-diagonal lhsT),
            offs  [N_CORES, 2K, PAIRS] f32 (pad = -1),
            iota  [2K, 256] f16,
            K (block K_pad; 2K <= 128 required for the HW path).
    """
    pf = np.asarray(pf, dtype=np.float32)
    vc = np.asarray(vc)
    b = vc[:, 0].astype(np.int64)
    y = vc[:, 2].astype(np.int64)
    x = vc[:, 3].astype(np.int64)
    cell = y * NX + x
    key = b * CELLS_B + cell

    # last occurrence of each key wins (matches reference scatter)
    u, idx_rev = np.unique(key[::-1], return_index=True)
    winners = (len(key) - 1) - idx_rev

    wb = u // CELLS_B
    wc = u % CELLS_B
    h = (wc >= HALF).astype(np.int64)
    core = wb * 2 + h
    cl = wc - h * HALF                      # 0..HALF-1 within slab

    main = cl < MAIN_PAIRS * 2 * TILE
    pair = np.where(main, cl // (2 * TILE), MAIN_PAIRS)
    j = np.where(main, cl % (2 * TILE), cl - MAIN_PAIRS * 2 * TILE)
    blk_sz = np.where(pair < MAIN_PAIRS, TILE, TAIL_COLS)
    blk = (j >= blk_sz).astype(np.int64)    # 0 = A, 1 = B
    off = j - blk * blk_sz                  # offset within block

    gkey = (core * PAIRS + pair) * 2 + blk
    order = np.argsort(gkey, kind="stable")
    gk_s = gkey[order]
    starts = np.r_[0, np.flatnonzero(np.diff(gk_s)) + 1]
    counts = np.diff(np.r_[starts, len(gk_s)])
    K = max(16, int(np.ceil(counts.max() / 16) * 16))

    rank = np.arange(len(gk_s)) - np.repeat(starts, counts)
    w_s = winners[order]
    core_s = core[order]
    pair_s = pair[order]
    blk_s = blk[order]
    off_s = off[order]

    KB = -(-K // 32) * 32            # B-block partition offset (32-aligned)
    feat = np.zeros((N_CORES, 2, K, PAIRS, 64), np.float16)
    offs = np.full((N_CORES, KB + K, PAIRS), -1.0, np.float32)
    feat[core_s, blk_s, rank, pair_s, :] = pf[w_s].astype(np.float16)
    offs[core_s, blk_s * KB + rank, pair_s] = off_s

    iota = np.broadcast_to(
        np.arange(TILE, dtype=np.float16)[None, :], (KB + K, TILE)
    ).copy()
    featA = feat[:, 0].reshape(N_CORES, K, PAIRS * 64)
    featB = feat[:, 1].reshape(N_CORES, K, PAIRS * 64)
    return featA, featB, offs, iota, K


def _unshuffle(out_dev):
    """[N_CORES, 128, OUT_COLS] -> [B, C, NY, NX]."""
    full = np.empty((B, C, CELLS_B), np.float32)
    for core in range(N_CORES):
        bb, hh = core // 2, core % 2
        od = out_dev[core]
        slab = np.empty((C, HALF), np.float32)
        m = MAIN_PAIRS * TILE
        s = slab[:, : 2 * m].reshape(C, MAIN_PAIRS, 2 * TILE)
        s[:, :, :TILE] = od[:64, :m].reshape(C, MAIN_PAIRS, TILE)
        s[:, :, TILE:] = od[64:, :m].reshape(C, MAIN_PAIRS, TILE)
        slab[:, 2 * m : 2 * m + TAIL_COLS] = od[:64, m:]
        slab[:, 2 * m + TAIL_COLS :] = od[64:, m:]
        full[bb, :, hh * HALF : (hh + 1) * HALF] = slab
    return full.reshape(B, C, NY, NX)


def _sim_core(featA_c, featB_c, offs_c, K):
    """Numpy simulation of one core's device program (for validation)."""
    KB = -(-K // 32) * 32
    out = np.zeros((128, OUT_COLS), np.float32)
    fA = featA_c.reshape(K, PAIRS, 64).astype(np.float32)
    fB = featB_c.reshape(K, PAIRS, 64).astype(np.float32)
    for t in range(PAIRS):
        n = TILE if t < MAIN_PAIRS else TAIL_COLS
        oh = (offs_c[:, t : t + 1] == np.arange(n)[None, :]).astype(np.float32)
        lhsT = np.zeros((KB + K, 128), np.float32)
        lhsT[:K, :64] = fA[:, t, :]
        lhsT[KB:, 64:] = fB[:, t, :]
        lo = t * TILE
        out[:, lo : lo + n] = lhsT.T @ oh
    return out


def _build_bass(K, dynamic=False, load_ring="scalar", split_feat="assemble", onehot="ts"):
    import concourse.bacc as bacc
    import concourse.tile as tile
    from concourse import mybir
    from contextlib import ExitStack

    f32 = mybir.dt.float32
    f16 = mybir.dt.float16
    i32 = mybir.dt.int32
    KB = -(-K // 32) * 32
    K2 = KB + K
    nc = bacc.Bacc("TRN2", target_bir_lowering=False, debug=False)

    # split_feat: "dense"    - host-built block-diag featT (5.2 MB loads)
    #             "dma"      - featA/featB strided-DMA'd into the diag blocks
    #             "assemble" - featA/featB contiguous loads + DVE copies
    if split_feat is True:
        split_feat = "dma"
    elif split_feat is False:
        split_feat = "dense"
    if split_feat == "assemble":
        featAB = nc.dram_tensor(
            "featAB", [K, 2 * PAIRS * 64], f16, kind="ExternalInput"
        )
    elif split_feat == "dma":
        featA = nc.dram_tensor("featA", [K, PAIRS * 64], f16, kind="ExternalInput")
        featB = nc.dram_tensor("featB", [K, PAIRS * 64], f16, kind="ExternalInput")
    else:
        featT = nc.dram_tensor(
            "featT", [K2, PAIRS * 128], f16, kind="ExternalInput"
        )
    offs = nc.dram_tensor(
        "offs", [K2, PAIRS], f32 if onehot == "ts" else f16, kind="ExternalInput"
    )
    iota = nc.dram_tensor("iota", [K2, TILE], f16, kind="ExternalInput")
    if dynamic:
        reps = nc.dram_tensor("reps", [1, 1], i32, kind="ExternalInput")
    out = nc.dram_tensor("out", [128, OUT_COLS], f32, kind="ExternalOutput")

    N_FEAT_BUFS = 3

    with tile.TileContext(nc) as tc, ExitStack() as ctx:
        const_p = ctx.enter_context(tc.tile_pool(name="const", bufs=1))
        feat_p = ctx.enter_context(tc.tile_pool(name="feat", bufs=N_FEAT_BUFS))
        oh_p = ctx.enter_context(tc.tile_pool(name="oh", bufs=16))
        ps_p = ctx.enter_context(tc.tile_pool(name="ps", bufs=2, space="PSUM"))
        st_p = ctx.enter_context(tc.tile_pool(name="st", bufs=4))
        ld_p = ctx.enter_context(tc.tile_pool(name="ld", bufs=N_FEAT_BUFS))

        iota_t = const_p.tile([K2, TILE], f16)
        nc.sync.dma_start(out=iota_t[:], in_=iota[:, :])
        off_t = const_p.tile([K2, PAIRS], f32 if onehot == "ts" else f16)
        nc.sync.dma_start(out=off_t[:], in_=offs[:, :])
        if dynamic:
            rt = const_p.tile([1, 1], i32)
            nc.sync.dma_start(out=rt[:], in_=reps[:, :])
            r_val = nc.values_load(
                rt[:], min_val=1, max_val=1 << 20, skip_runtime_bounds_check=True
            )

        # prologue: zero the anti-diagonal blocks of every feat buffer once;
        # the per-chunk DMAs below only ever write the diagonal blocks, so
        # the zeros persist across all loop iterations / buffer reuse.
        plan = _stage_plan(ramp=True)
        max_pairs = max(n for _, n in plan)
        if split_feat != "dense":
            for _ in range(N_FEAT_BUFS):
                zt = feat_p.tile([K2, max_pairs * 128], f16, tag="feat")
                nc.vector.memset(zt[:], 0)

        load_eng = {"scalar": nc.scalar, "sync": nc.sync, "gpsimd": nc.gpsimd}[
            load_ring
        ]

        def body():
            ab_col = 0
            for p0, n_pairs in plan:
                g_cols = _stage_cols(p0, n_pairs)
                c0 = p0 * TILE

                feat_chunk = feat_p.tile([K2, max_pairs * 128], f16, tag="feat")
                fc = feat_chunk[:, : n_pairs * 128]
                nA = n_pairs * 64
                if split_feat == "dma":
                    load_eng.dma_start(
                        out=fc[:K].rearrange("k (p c) -> k p c", p=n_pairs)[
                            :, :, :64
                        ],
                        in_=featA[:, p0 * 64 : (p0 + n_pairs) * 64],
                    )
                    load_eng.dma_start(
                        out=fc[KB:].rearrange("k (p c) -> k p c", p=n_pairs)[
                            :, :, 64:
                        ],
                        in_=featB[:, p0 * 64 : (p0 + n_pairs) * 64],
                    )
                elif split_feat == "assemble":
                    ld = ld_p.tile([K, 2 * max_pairs * 64], f16, tag="ld")
                    load_eng.dma_start(
                        out=ld[:, : 2 * nA], in_=featAB[:, ab_col : ab_col + 2 * nA]
                    )
                    nc.vector.tensor_scalar(
                        out=fc[:K].rearrange("k (p c) -> k p c", p=n_pairs)[
                            :, :, :64
                        ],
                        in0=ld[:, :nA].rearrange("k (p c) -> k p c", p=n_pairs),
                        scalar1=1.0,
                        scalar2=None,
                        op0=mybir.AluOpType.mult,
                    )
                    nc.vector.tensor_scalar(
                        out=fc[KB:].rearrange("k (p c) -> k p c", p=n_pairs)[
                            :, :, 64:
                        ],
                        in0=ld[:, nA : 2 * nA].rearrange(
                            "k (p c) -> k p c", p=n_pairs
                        ),
                        scalar1=1.0,
                        scalar2=None,
                        op0=mybir.AluOpType.mult,
                    )
                else:
                    load_eng.dma_start(
                        out=fc[:],
                        in_=featT[:, p0 * 128 : (p0 + n_pairs) * 128],
                    )

                stage = st_p.tile([128, g_cols], f32, tag="st")
                n_ps = (n_pairs + PAIRS_PER_PSUM - 1) // PAIRS_PER_PSUM
                for q in range(n_ps):
                    qp0 = q * PAIRS_PER_PSUM
                    q_pairs = min(PAIRS_PER_PSUM, n_pairs - qp0)
                    q_cols = _stage_cols(p0 + qp0, q_pairs)
                    psum = ps_p.tile([128, q_cols], f32, tag="ps")
                    col = 0
                    for i in range(q_pairs):
                        t = p0 + qp0 + i
                        n = TILE if t < MAIN_PAIRS else TAIL_COLS
                        oh = oh_p.tile([K2, n], f16, tag="oh")
                        if onehot == "ts":
                            nc.vector.tensor_scalar(
                                out=oh[:],
                                in0=iota_t[:, :n],
                                scalar1=off_t[:, t : t + 1],
                                scalar2=None,
                                op0=mybir.AluOpType.is_equal,
                            )
                        else:
                            nc.vector.tensor_tensor(
                                out=oh[:],
                                in0=off_t[:, t : t + 1].to_broadcast([K2, n]),
                                in1=iota_t[:, :n],
                                op=mybir.AluOpType.is_equal,
                            )
                        j = qp0 + i
                        nc.tensor.matmul(
                            out=psum[:, col : col + n],
                            lhsT=feat_chunk[:, j * 128 : j * 128 + 128],
                            rhs=oh[:],
                            is_transpose=False,
                            start=True,
                            stop=True,
                        )
                        col += n
                    nc.scalar.copy(
                        out=stage[:, qp0 * TILE : qp0 * TILE + q_cols], in_=psum[:]
                    )
                    # alternate the two HWDGE rings to spread store-queue
                    # pressure (loads share the scalar ring)
                    (nc.sync if (p0 + qp0) % 16 < 8 else nc.scalar).dma_start(
                        out=out[:, c0 + qp0 * TILE : c0 + qp0 * TILE + q_cols],
                        in_=stage[:, qp0 * TILE : qp0 * TILE + q_cols],
                    )
                ab_col += 2 * nA

        if dynamic:
            with tc.For_i(0, r_val, 1):
                body()
        else:
            body()

    nc.compile()
    return nc


def _dense_featT(featA, featB, K):
    """[N_CORES, KB+K, PAIRS*128] block-diagonal fp16 (for dense loads)."""
    KB = -(-K // 32) * 32
    featT = np.zeros((N_CORES, KB + K, PAIRS, 128), np.float16)
    featT[:, :K, :, :64] = featA.reshape(N_CORES, K, PAIRS, 64)
    featT[:, KB:, :, 64:] = featB.reshape(N_CORES, K, PAIRS, 64)
    return featT.reshape(N_CORES, KB + K, PAIRS * 128)


def _make_in_maps(
    featA, featB, offs, iota, dynamic=False, split_feat="assemble", K=None, onehot="ts"
):
    if split_feat is True:
        split_feat = "dma"
    elif split_feat is False:
        split_feat = "dense"
    dense = _dense_featT(featA, featB, K) if split_feat == "dense" else None
    fab = (
        _featAB(featA, featB, K, _stage_plan(ramp=True))
        if split_feat == "assemble"
        else None
    )
    offs_in = offs if onehot == "ts" else offs.astype(np.float16)
    maps = []
    for c in range(N_CORES):
        if split_feat == "assemble":
            m = {"featAB": fab[c]}
        elif split_feat == "dma":
            m = {"featA": featA[c], "featB": featB[c]}
        else:
            m = {"featT": dense[c]}
        m.update({"offs": offs_in[c], "iota": iota})
        if dynamic:
            m["reps"] = np.ones((1, 1), np.int32)
        maps.append(m)
    return maps


def _run(featA, featB, offs, iota, K):
    from concourse.bass_utils import run_bass_kernel_spmd

    nc = _build_bass(K)
    in_maps = _make_in_maps(featA, featB, offs, iota, K=K)
    res = run_bass_kernel_spmd(nc, in_maps, core_ids=list(range(N_CORES)))
    out_dev = np.stack([res.results[c]["out"] for c in range(N_CORES)])
    return _unshuffle(out_dev)


def kernel(pillar_features, voxel_coords):
    featA, featB, offs, iota, K = _host_prep(pillar_features, voxel_coords)
    if K > 64:
        # PE contraction is capped at 128 partitions; with the given input
        # distribution K is 48, so this path is never taken. Correctness
        # safety net only.
        out_dev = np.stack(
            [_sim_core(featA[c], featB[c], offs[c], K) for c in range(N_CORES)]
        )
        return _unshuffle(out_dev)
    return _run(featA, featB, offs, iota, K)


if __name__ == "__main__":
    # numpy-sim self check against last-wins reference
    rng = np.random.default_rng(0)
    n = 100000
    pf = rng.standard_normal((n, 64)).astype(np.float32)
    vc = np.stack(
        [
            rng.integers(0, B, n),
            np.zeros(n, np.int64),
            rng.integers(0, NY, n),
            rng.integers(0, NX, n),
        ],
        axis=1,
    ).astype(np.int64)
    featA, featB, offs, iota, K = _host_prep(pf, vc)
    print("K =", K)
    out_dev = np.stack(
        [_sim_core(featA[c], featB[c], offs[c], K) for c in range(N_CORES)]
    )
    got = _unshuffle(out_dev)
    grid = np.zeros((B * CELLS_B, 64), np.float32)
    flat = vc[:, 0] * CELLS_B + vc[:, 2] * NX + vc[:, 3]
    grid[flat] = pf
    ref = grid.reshape(B, CELLS_B, 64).transpose(0, 2, 1).reshape(B, C, NY, NX)
    err = np.abs(got - ref).max() / np.abs(ref).max()
    print("max rel diff vs f32 last-wins reference:", err)
    assert err < 1e-3, err
    print("numpy sim matches (up to fp16 rounding)")
